# revision 2
# baseline (speedup 1.0000x reference)
"""AttnMPNN (GNN message passing w/ edge softmax) on 8 Trainium2 NeuronCores.

v3 "dst-mod-128" design:
  - 8 cores each own 6250 destination nodes, padded to 6272 = 49 slots x 128.
  - Node n of a core is assigned a (slot s, partition p) position by a
    degree-sorted packing (lexsort by per-half in-degree) so that per-slot
    max in-degree is close to mean (padding ~1.22x).
  - All in-edges of node (s, p) live on PARTITION p of slot s, along the
    free dim. Segment-sum becomes a free-axis tree reduce; the dst-side
    linear term B~[dst] is a per-partition broadcast. No one-hot selection
    matrices, no expansion/accumulation matmuls.
  - wfc folded into the attn weights on host (A~ = A*wfc, B~ = B*wfc with
    hidden dims permuted positives-first) so logits = sum(max(y,0)[:kp])
    - sum(relu(-y)[kp:]), computed with 2 ACT relus + 2 DVE reduces.
  - C table = [A~ | nf] rows (256B) in DRAM, 2 halves of HR rows each so
    gather indices fit int16. Row 0 of each half is a PAD row with
    A~ = -/+3000 (sign-matched so z == 0 exactly -> wt == 1.0 exactly) and
    nf = 0; wsum is corrected by subtracting the host-computed pad count.
  - Softmax max-subtraction dropped (logits are O(1); baseline validated).
"""

import numpy as np

P = 128
D = 64
SPLIT = 25088   # node-id boundary between the two C-table halves
HR = 25216      # rows per half: 128 pad rows + up to 25088 node rows
NPC = 6250
SLOTS = 49
NPAD = SLOTS * P
N_NODES = 50000


def _wrap16(vec):
    """[n] int16 -> [128, n//16] wrapped by 16, replicated to 8 groups."""
    n = vec.shape[0]
    w = vec.reshape(n // 16, 16).T  # [16, n//16]
    return np.ascontiguousarray(np.tile(w, (8, 1)))


def _build_program(cfg):
    import concourse.bass as bass  # noqa: F401
    import concourse.tile as tile
    from concourse import bacc, mybir

    K0 = cfg["K0"]
    K1 = cfg["K1"]
    KP = cfg["KP"]   # number of positive-wfc hidden dims
    KM0 = max(K0)
    KM1 = max(K1)
    KMAX = max(k0 + k1 for k0, k1 in zip(K0, K1))
    OFF0 = np.concatenate([[0], np.cumsum(np.array(K0) * 8)])
    OFF1 = np.concatenate([[0], np.cumsum(np.array(K1) * 8)])
    NFC = 2 * HR

    f32 = mybir.dt.float32
    bf16 = mybir.dt.bfloat16
    i16 = mybir.dt.int16

    _AFT = mybir.ActivationFunctionType
    _ALT = mybir.AxisListType
    _ALU = mybir.AluOpType

    nc = bacc.Bacc("TRN2", target_bir_lowering=False, debug=False,
                   enable_asserts=False)

    t_nftC = nc.dram_tensor("nftC", (D + 1, NFC), bf16, kind="ExternalInput")
    t_nfbC = nc.dram_tensor("nfbC", (NFC, D), bf16, kind="ExternalInput")
    t_padC = nc.dram_tensor("padC", (P, 2 * D), bf16, kind="ExternalInput")
    t_nfto = nc.dram_tensor("nfto", (D + 1, NPAD), bf16, kind="ExternalInput")
    t_w1 = nc.dram_tensor("w1", (D + 1, D), bf16, kind="ExternalInput")
    t_w2 = nc.dram_tensor("w2", (D + 1, D), bf16, kind="ExternalInput")
    t_wn1 = nc.dram_tensor("wn1", (D, D), bf16, kind="ExternalInput")
    t_wn2 = nc.dram_tensor("wn2", (D, D), bf16, kind="ExternalInput")
    t_bn = nc.dram_tensor("bn", (D,), f32, kind="ExternalInput")
    t_i0 = nc.dram_tensor("i0", (P, int(OFF0[-1])), i16, kind="ExternalInput")
    t_i1 = nc.dram_tensor("i1", (P, int(OFF1[-1])), i16, kind="ExternalInput")
    t_npad = nc.dram_tensor("npad", (P, SLOTS), f32, kind="ExternalInput")
    t_dginv = nc.dram_tensor("dginv", (P, SLOTS), f32, kind="ExternalInput")
    t_C = nc.dram_tensor("C_scr", (NFC, 2 * D), bf16, kind="ExternalOutput")
    t_out = nc.dram_tensor("out", (NPAD, D), f32, kind="ExternalOutput")

    with tile.TileContext(nc) as tc:
        import contextlib
        ctx = contextlib.ExitStack()
        with ctx:
            const_p = ctx.enter_context(tc.tile_pool(name="const", bufs=1))

            from concourse.masks import make_identity
            ident = const_p.tile([P, P], f32)
            make_identity(nc, ident[:])
            w1_sb = const_p.tile([D + 1, D], bf16)
            nc.sync.dma_start(w1_sb[:], t_w1.ap())
            w2_sb = const_p.tile([D + 1, D], bf16)
            nc.sync.dma_start(w2_sb[:], t_w2.ap())
            wn1_sb = const_p.tile([D, D], bf16)
            nc.sync.dma_start(wn1_sb[:], t_wn1.ap())
            wn2_sb = const_p.tile([D, D], bf16)
            nc.sync.dma_start(wn2_sb[:], t_wn2.ap())
            bn_b = const_p.tile([P, D], f32)
            nc.sync.dma_start(bn_b[:], t_bn.ap()[None, :].broadcast_to((P, D)))
            npad_sb = const_p.tile([P, SLOTS], f32)
            nc.sync.dma_start(npad_sb[:], t_npad.ap())
            dginv_sb = const_p.tile([P, SLOTS], f32)
            nc.sync.dma_start(dginv_sb[:], t_dginv.ap())
            nfto_sb = const_p.tile([D + 1, NPAD], bf16)
            nc.sync.dma_start(nfto_sb[:], t_nfto.ap())
            padC_sb = const_p.tile([P, 2 * D], bf16)
            nc.sync.dma_start(padC_sb[:], t_padC.ap())
            Bt_sb = const_p.tile([P, SLOTS, D], bf16)

            # ---- phase 1: C = [A~ | nf] to DRAM; B~ (own, permuted) in SBUF
            with contextlib.ExitStack() as pctx:
                pre_p = pctx.enter_context(tc.tile_pool(name="pre", bufs=3))
                pre_ps = pctx.enter_context(
                    tc.tile_pool(name="preps", bufs=2, space="PSUM"))
                # pad rows at the head of each half
                nc.sync.dma_start(t_C.ap()[0:P, :], padC_sb[:])
                nc.sync.dma_start(t_C.ap()[HR:HR + P, :], padC_sb[:])
                GB = 8
                # compute blocks: rows [128, HR) of each half
                NBH = (HR - P) // P  # 196
                for h in (0, 1):
                    base = h * HR + P
                    for g in range((NBH + GB - 1) // GB):
                        nb = min(GB, NBH - g * GB)
                        c0 = base + g * GB * P
                        xt = pre_p.tile([D + 1, GB * P], bf16, tag="xt")
                        nc.sync.dma_start(
                            xt[:, :nb * P], t_nftC.ap()[:, c0:c0 + nb * P])
                        ps = pre_ps.tile([P, GB, D], f32, tag="ps")
                        for j in range(nb):
                            nc.tensor.matmul(
                                out=ps[:, j, :], lhsT=xt[:, j * P:(j + 1) * P],
                                rhs=w1_sb[:], start=True, stop=True)
                        cb = pre_p.tile([P, GB, 2 * D], bf16, tag="cb")
                        nc.scalar.copy(cb[:, :nb, :D], ps[:, :nb, :])
                        nc.sync.dma_start(
                            cb[:, :nb, D:],
                            t_nfbC.ap()[c0:c0 + nb * P, :].rearrange(
                                "(b p) d -> p b d", p=P))
                        nc.sync.dma_start(
                            t_C.ap()[c0:c0 + nb * P, :].rearrange(
                                "(b p) d -> p b d", p=P), cb[:, :nb, :])
                # B~ for own nodes in permuted (slot, partition) order
                for g in range((SLOTS + GB - 1) // GB):
                    nb = min(GB, SLOTS - g * GB)
                    c0 = g * GB * P
                    ps = pre_ps.tile([P, GB, D], f32, tag="bps")
                    for j in range(nb):
                        nc.tensor.matmul(
                            out=ps[:, j, :],
                            lhsT=nfto_sb[:, c0 + j * P:c0 + (j + 1) * P],
                            rhs=w2_sb[:], start=True, stop=True)
                    nc.scalar.copy(
                        Bt_sb[:, g * GB:g * GB + nb, :], ps[:, :nb, :])

            # ---- phase 2: per-slot edge processing
            i_p = ctx.enter_context(tc.tile_pool(name="idx", bufs=8))
            g_p = ctx.enter_context(tc.tile_pool(name="gath", bufs=6))
            y_p = ctx.enter_context(tc.tile_pool(name="ybuf", bufs=3))
            z_p = ctx.enter_context(tc.tile_pool(name="zbuf", bufs=3))
            pay_p = ctx.enter_context(tc.tile_pool(name="pay", bufs=3))
            s_p = ctx.enter_context(tc.tile_pool(name="small", bufs=4))
            fin_p = ctx.enter_context(tc.tile_pool(name="fin", bufs=3))
            fin_ps = ctx.enter_context(
                tc.tile_pool(name="finps", bufs=2, space="PSUM"))

            for s in range(SLOTS):
                k0, k1 = K0[s], K1[s]
                k = k0 + k1
                gC = g_p.tile([P, KMAX, 2 * D], bf16, tag="gC")
                if k0 > 0:
                    i0 = i_p.tile([P, KM0 * 8], i16, tag="i0")
                    nc.sync.dma_start(
                        i0[:, :k0 * 8],
                        t_i0.ap()[:, int(OFF0[s]):int(OFF0[s]) + k0 * 8])
                    nc.gpsimd.dma_gather(
                        out_ap=gC[:, :k0, :], in_ap=t_C.ap()[:HR, :],
                        idxs_ap=i0[:, :k0 * 8], num_idxs=k0 * P,
                        num_idxs_reg=k0 * P, elem_size=2 * D,
                        single_packet=False)
                if k1 > 0:
                    i1 = i_p.tile([P, KM1 * 8], i16, tag="i1")
                    nc.sync.dma_start(
                        i1[:, :k1 * 8],
                        t_i1.ap()[:, int(OFF1[s]):int(OFF1[s]) + k1 * 8])
                    nc.gpsimd.dma_gather(
                        out_ap=gC[:, k0:k, :], in_ap=t_C.ap()[HR:, :],
                        idxs_ap=i1[:, :k1 * 8], num_idxs=k1 * P,
                        num_idxs_reg=k1 * P, elem_size=2 * D,
                        single_packet=False)

                # y = A~[src] + B~[dst]  (dst == partition's own node)
                y = y_p.tile([P, KMAX, D], bf16, tag="y")
                nc.vector.tensor_tensor(
                    out=y[:, :k, :], in0=gC[:, :k, :D],
                    in1=Bt_sb[:, s, :].unsqueeze(1).broadcast_to((P, k, D)),
                    op=_ALU.add)
                # z: positives-first clamp on ACT engine
                z = z_p.tile([P, KMAX, D], bf16, tag="z")
                if KP > 0:
                    nc.scalar.activation(z[:, :k, :KP], y[:, :k, :KP],
                                         func=_AFT.Relu)
                if KP < D:
                    nc.scalar.activation(z[:, :k, KP:], y[:, :k, KP:],
                                         func=_AFT.Relu, scale=-1.0)
                # logits = sum(z_pos) - sum(z_negstore)
                lg = s_p.tile([P, KMAX], f32, tag="lg")
                if KP > 0 and KP < D:
                    r1 = s_p.tile([P, KMAX], f32, tag="r1")
                    nc.vector.tensor_reduce(lg[:, :k], z[:, :k, :KP],
                                            axis=_ALT.X, op=_ALU.add)
                    nc.vector.tensor_reduce(r1[:, :k], z[:, :k, KP:],
                                            axis=_ALT.X, op=_ALU.add)
                    nc.vector.tensor_tensor(out=lg[:, :k], in0=lg[:, :k],
                                            in1=r1[:, :k], op=_ALU.subtract)
                elif KP == D:
                    nc.vector.tensor_reduce(lg[:, :k], z[:, :k, :],
                                            axis=_ALT.X, op=_ALU.add)
                else:
                    nc.vector.tensor_reduce(lg[:, :k], z[:, :k, :],
                                            axis=_ALT.X, op=_ALU.add)
                    nc.vector.tensor_scalar_mul(lg[:, :k], lg[:, :k], -1.0)
                # wt = exp(leaky_relu(lg))
                lk = s_p.tile([P, KMAX], f32, tag="lk")
                nc.scalar.activation(lk[:, :k], lg[:, :k], func=_AFT.Lrelu,
                                     alpha=0.01)
                wt = s_p.tile([P, KMAX], bf16, tag="wt")
                nc.scalar.activation(wt[:, :k], lk[:, :k], func=_AFT.Exp)

                # pay = nf[src] * wt ; tree-reduce over k -> agg
                pay = pay_p.tile([P, KMAX, D], f32, tag="pay")
                nc.vector.tensor_tensor(
                    out=pay[:, :k, :], in0=gC[:, :k, D:],
                    in1=wt[:, :k].unsqueeze(2).broadcast_to((P, k, D)),
                    op=_ALU.mult)
                m = k
                while m > 1:
                    a = m // 2
                    nc.vector.tensor_tensor(
                        out=pay[:, :a, :], in0=pay[:, :a, :],
                        in1=pay[:, m - a:m, :], op=_ALU.add)
                    m -= a

                # wsum (pad-corrected), normalize
                ws = fin_p.tile([P, 1], f32, tag="ws")
                nc.vector.tensor_reduce(ws[:], wt[:, :k], axis=_ALT.X,
                                        op=_ALU.add)
                nc.vector.tensor_tensor(out=ws[:], in0=ws[:],
                                        in1=npad_sb[:, s:s + 1],
                                        op=_ALU.subtract)
                den = fin_p.tile([P, 1], f32, tag="den")
                nc.vector.tensor_scalar_max(den[:], ws[:], 1e-30)
                rec = fin_p.tile([P, 1], f32, tag="rec")
                nc.vector.reciprocal(rec[:], den[:])
                rec2 = fin_p.tile([P, 1], f32, tag="rec2")
                nc.vector.tensor_mul(rec2[:], rec[:], dginv_sb[:, s:s + 1])
                am = fin_p.tile([P, D], f32, tag="am")
                nc.vector.tensor_mul(
                    am[:], pay[:, 0, :], rec2[:].broadcast_to((P, D)))

                # finale: out = [nf_own | agg] @ Wn + bn
                amT_ps = fin_ps.tile([D, P], f32, tag="amT")
                nc.tensor.transpose(out=amT_ps[:], in_=am[:], identity=ident[:])
                amT = fin_p.tile([D, P], bf16, tag="amTs")
                nc.vector.tensor_copy(amT[:], amT_ps[:])
                o_ps = fin_ps.tile([P, D], f32, tag="ops")
                nc.tensor.matmul(out=o_ps[:],
                                 lhsT=nfto_sb[:D, s * P:(s + 1) * P],
                                 rhs=wn1_sb[:], start=True, stop=False)
                nc.tensor.matmul(out=o_ps[:], lhsT=amT[:], rhs=wn2_sb[:],
                                 start=False, stop=True)
                o_sb = fin_p.tile([P, D], f32, tag="osb")
                nc.vector.tensor_add(o_sb[:], o_ps[:], bn_b[:])
                nc.sync.dma_start(t_out.ap()[s * P:(s + 1) * P, :], o_sb[:])

    nc.compile()
    return nc


_PROGRAM_CACHE = {}


def _get_program(cfg_key, cfg):
    if cfg_key not in _PROGRAM_CACHE:
        _PROGRAM_CACHE[cfg_key] = _build_program(cfg)
    return _PROGRAM_CACHE[cfg_key]


def _prep(nf, src, dst, W_attn, b_attn, w_fc, W_node, b_node, n_cores=8):
    from ml_dtypes import bfloat16

    N = nf.shape[0]
    assert N == N_NODES
    src = np.asarray(src).astype(np.int64)
    dst = np.asarray(dst).astype(np.int64)
    nf = np.asarray(nf, dtype=np.float32)
    W_attn = np.asarray(W_attn, dtype=np.float32)
    b_attn = np.asarray(b_attn, dtype=np.float32)
    w_fc = np.asarray(w_fc, dtype=np.float32)
    W_node = np.asarray(W_node, dtype=np.float32)
    b_node = np.asarray(b_node, dtype=np.float32)

    # hidden-dim permutation: positive wfc dims first; fold wfc into W_attn
    permh = np.argsort(w_fc < 0, kind="stable")
    wfcP = w_fc[permh]
    KP = int((wfcP >= 0).sum())
    w1t = np.concatenate([W_attn[:D], b_attn[None, :]], 0)[:, permh] * wfcP
    w2t = np.concatenate([W_attn[D:], np.zeros((1, D), np.float32)], 0)[
        :, permh] * wfcP

    half = (src >= SPLIT).astype(np.int64)
    src_local = np.where(half == 0, src + P, src - SPLIT + P)

    deg = np.bincount(dst, minlength=N)
    h0 = np.bincount(dst[half == 0], minlength=N)
    h1 = deg - h0

    owner = dst // NPC

    # per-core node packing (perm: rank -> local node id)
    perms = []
    pos_s = np.zeros(N, np.int64)
    pos_p = np.zeros(N, np.int64)
    for c in range(n_cores):
        a = h0[c * NPC:(c + 1) * NPC]
        b = h1[c * NPC:(c + 1) * NPC]
        order = np.lexsort((-b, -a))  # rank -> local node
        perms.append(order)
        nodes = c * NPC + order
        ranks = np.arange(NPC)
        pos_s[nodes] = ranks // P
        pos_p[nodes] = ranks % P

    s_e = pos_s[dst]
    p_e = pos_p[dst]

    # per (core, slot, partition, half) counts -> K0/K1 (shared across cores)
    cnt = np.zeros((n_cores, SLOTS, P, 2), np.int64)
    np.add.at(cnt, (owner, s_e, p_e, half), 1)
    K0 = cnt[:, :, :, 0].max(axis=(0, 2))
    K1 = cnt[:, :, :, 1].max(axis=(0, 2))
    K0 = np.maximum(K0, 1).astype(np.int64)
    K1 = np.maximum(K1, 1).astype(np.int64)

    # rank of each edge within its (dst, half) group
    key = ((owner * SLOTS + s_e) * P + p_e) * 2 + half
    order_e = np.argsort(key, kind="stable")
    skey = key[order_e]
    starts = np.searchsorted(skey, np.arange(n_cores * SLOTS * P * 2 + 1))
    rank_in_grp = np.arange(len(src)) - starts[skey]

    OFF0 = np.concatenate([[0], np.cumsum(K0)])
    OFF1 = np.concatenate([[0], np.cumsum(K1)])
    T0, T1 = int(OFF0[-1]), int(OFF1[-1])

    # flat per-core idx vectors (0 = pad row)
    v0 = np.zeros((n_cores, T0 * P), np.int16)
    v1 = np.zeros((n_cores, T1 * P), np.int16)
    oe_src = src_local[order_e]
    oe_half = half[order_e]
    oe_owner = owner[order_e]
    oe_s = s_e[order_e]
    oe_p = p_e[order_e]
    m0 = oe_half == 0
    pos0 = (OFF0[oe_s[m0]] + rank_in_grp[m0]) * P + oe_p[m0]
    v0[oe_owner[m0], pos0] = oe_src[m0]
    m1 = ~m0
    pos1 = (OFF1[oe_s[m1]] + rank_in_grp[m1]) * P + oe_p[m1]
    v1[oe_owner[m1], pos1] = oe_src[m1]

    # pad counts / deg inverse per (p, s), permuted order
    npad = (K0[None, :, None] - cnt[:, :, :, 0]
            + K1[None, :, None] - cnt[:, :, :, 1]).astype(np.float32)
    npad = npad.transpose(0, 2, 1)  # [core, p, s]

    cfg = {"K0": tuple(int(x) for x in K0), "K1": tuple(int(x) for x in K1),
           "KP": KP}

    # C-table-ordered node features
    NFC = 2 * HR
    nftC = np.zeros((D + 1, NFC), np.float32)
    nftC[D, :] = 1.0
    nftC[:D, P:P + SPLIT] = nf[:SPLIT].T
    nftC[:D, HR + P:HR + P + (N - SPLIT)] = nf[SPLIT:].T
    nfbC = np.zeros((NFC, D), np.float32)
    nfbC[P:P + SPLIT] = nf[:SPLIT]
    nfbC[HR + P:HR + P + (N - SPLIT)] = nf[SPLIT:]
    padC = np.zeros((P, 2 * D), np.float32)
    padC[:, :KP] = -3000.0
    padC[:, KP:D] = 3000.0

    in_maps = []
    for c in range(n_cores):
        order = perms[c]
        nodes = c * NPC + order
        nfto = np.zeros((D + 1, NPAD), np.float32)
        nfto[D, :] = 1.0
        nfto[:D, :NPC] = nf[nodes].T
        degc = np.ones(NPAD, np.float32)
        degc[:NPC] = np.maximum(deg[nodes], 1.0)
        dginv = np.ascontiguousarray(
            (1.0 / degc).reshape(SLOTS, P).T).astype(np.float32)
        i0 = np.concatenate(
            [_wrap16(v0[c, OFF0[s] * P:OFF0[s + 1] * P])
             for s in range(SLOTS)], axis=1)
        i1 = np.concatenate(
            [_wrap16(v1[c, OFF1[s] * P:OFF1[s + 1] * P])
             for s in range(SLOTS)], axis=1)
        in_maps.append({
            "nftC": np.ascontiguousarray(nftC).astype(bfloat16),
            "nfbC": np.ascontiguousarray(nfbC).astype(bfloat16),
            "padC": padC.astype(bfloat16),
            "nfto": np.ascontiguousarray(nfto).astype(bfloat16),
            "w1": np.ascontiguousarray(w1t).astype(bfloat16),
            "w2": np.ascontiguousarray(w2t).astype(bfloat16),
            "wn1": np.ascontiguousarray(W_node[:D]).astype(bfloat16),
            "wn2": np.ascontiguousarray(W_node[D:]).astype(bfloat16),
            "bn": b_node.astype(np.float32),
            "i0": i0, "i1": i1,
            "npad": np.ascontiguousarray(npad[c]),
            "dginv": dginv,
        })
    return cfg, in_maps, perms


def _run(inputs, trace=False):
    import concourse.bass_utils as bass_utils

    cfg, in_maps, perms = _prep(**inputs)
    cfg_key = (cfg["K0"], cfg["K1"], cfg["KP"])
    nc = _get_program(cfg_key, cfg)
    res = bass_utils.run_bass_kernel_spmd(nc, in_maps,
                                          core_ids=list(range(8)),
                                          trace=trace)
    out = np.empty((N_NODES, D), np.float32)
    for c in range(8):
        rows = np.asarray(res.results[c]["out"][:NPC], np.float32)
        out[c * NPC + perms[c]] = rows
    return out, res


def kernel(nf, src, dst, W_attn, b_attn, w_fc, W_node, b_node):
    out, _ = _run(dict(nf=nf, src=src, dst=dst, W_attn=W_attn, b_attn=b_attn,
                       w_fc=w_fc, W_node=W_node, b_node=b_node))
    return out


# revision 6
# speedup vs baseline: 1.4734x; 1.4734x over previous
"""AttnMPNN (GNN message passing w/ edge softmax) on 8 Trainium2 NeuronCores.

v3 "dst-mod-128" design:
  - 8 cores each own 6250 destination nodes, padded to 6272 = 49 slots x 128.
  - Node n of a core is assigned a (slot s, partition p) position by a
    degree-sorted packing (lexsort by per-half in-degree) so that per-slot
    max in-degree is close to mean (padding ~1.22x).
  - All in-edges of node (s, p) live on PARTITION p of slot s, along the
    free dim. Segment-sum becomes a free-axis tree reduce; the dst-side
    linear term B~[dst] is a per-partition broadcast. No one-hot selection
    matrices, no expansion/accumulation matmuls.
  - wfc folded into the attn weights on host (A~ = A*wfc, B~ = B*wfc with
    hidden dims permuted positives-first) so logits = sum(max(y,0)[:kp])
    - sum(relu(-y)[kp:]), computed with 2 ACT relus + 2 DVE reduces.
  - C table = [A~ | nf] rows (256B) in DRAM, 2 halves of HR rows each so
    gather indices fit int16. Row 0 of each half is a PAD row with
    A~ = -/+3000 (sign-matched so z == 0 exactly -> wt == 1.0 exactly) and
    nf = 0; wsum is corrected by subtracting the host-computed pad count.
  - Softmax max-subtraction dropped (logits are O(1); baseline validated).
"""

import numpy as np

P = 128
D = 64
SPLIT = 25088   # node-id boundary between the two C-table halves
HR = 25216      # rows per half: 128 pad rows + up to 25088 node rows
NPC = 6250
SLOTS = 49
NPAD = SLOTS * P
N_NODES = 50000


def _wrap16(vec):
    """[n] int16 -> [128, n//16] wrapped by 16, replicated to 8 groups."""
    n = vec.shape[0]
    w = vec.reshape(n // 16, 16).T  # [16, n//16]
    return np.ascontiguousarray(np.tile(w, (8, 1)))


def _build_program(cfg):
    import concourse.bass as bass  # noqa: F401
    import concourse.tile as tile
    from concourse import bacc, mybir

    K0 = cfg["K0"]
    K1 = cfg["K1"]
    KP = cfg["KP"]   # number of positive-wfc hidden dims
    KM0 = max(K0)
    KM1 = max(K1)
    KMAX = max(k0 + k1 for k0, k1 in zip(K0, K1))
    OFF0 = np.concatenate([[0], np.cumsum(np.array(K0) * 8)])
    OFF1 = np.concatenate([[0], np.cumsum(np.array(K1) * 8)])
    NFC = 2 * HR

    f32 = mybir.dt.float32
    bf16 = mybir.dt.bfloat16
    i16 = mybir.dt.int16

    _AFT = mybir.ActivationFunctionType
    _ALT = mybir.AxisListType
    _ALU = mybir.AluOpType

    nc = bacc.Bacc("TRN2", target_bir_lowering=False, debug=False,
                   enable_asserts=False, num_swdge_queues=4,
                   dynamic_dma_scratch_size=65536)

    t_nftC = nc.dram_tensor("nftC", (D + 1, NFC), bf16, kind="ExternalInput")
    t_nfbC = nc.dram_tensor("nfbC", (NFC, D), bf16, kind="ExternalInput")
    t_padC = nc.dram_tensor("padC", (P, 2 * D), bf16, kind="ExternalInput")
    t_nfto = nc.dram_tensor("nfto", (D + 1, NPAD), bf16, kind="ExternalInput")
    t_w1 = nc.dram_tensor("w1", (D + 1, D), bf16, kind="ExternalInput")
    t_w2 = nc.dram_tensor("w2", (D + 1, D), bf16, kind="ExternalInput")
    t_wn1 = nc.dram_tensor("wn1", (D, D), bf16, kind="ExternalInput")
    t_wn2 = nc.dram_tensor("wn2", (D, D), bf16, kind="ExternalInput")
    t_bn = nc.dram_tensor("bn", (D,), f32, kind="ExternalInput")
    t_i0 = nc.dram_tensor("i0", (P, int(OFF0[-1])), i16, kind="ExternalInput")
    t_i1 = nc.dram_tensor("i1", (P, int(OFF1[-1])), i16, kind="ExternalInput")
    t_npad = nc.dram_tensor("npad", (P, SLOTS), f32, kind="ExternalInput")
    t_dginv = nc.dram_tensor("dginv", (P, SLOTS), f32, kind="ExternalInput")
    t_C = nc.dram_tensor("C_scr", (NFC, 2 * D), bf16, kind="ExternalOutput")
    t_out = nc.dram_tensor("out", (NPAD, D), f32, kind="ExternalOutput")

    with tile.TileContext(nc) as tc:
        import contextlib
        ctx = contextlib.ExitStack()
        with ctx:
            const_p = ctx.enter_context(tc.tile_pool(name="const", bufs=1))

            from concourse.masks import make_identity
            ident = const_p.tile([P, P], f32)
            make_identity(nc, ident[:])
            w1_sb = const_p.tile([D + 1, D], bf16)
            nc.sync.dma_start(w1_sb[:], t_w1.ap())
            w2_sb = const_p.tile([D + 1, D], bf16)
            nc.sync.dma_start(w2_sb[:], t_w2.ap())
            wn1_sb = const_p.tile([D, D], bf16)
            nc.sync.dma_start(wn1_sb[:], t_wn1.ap())
            wn2_sb = const_p.tile([D, D], bf16)
            nc.sync.dma_start(wn2_sb[:], t_wn2.ap())
            bn_b = const_p.tile([P, D], f32)
            nc.sync.dma_start(bn_b[:], t_bn.ap()[None, :].broadcast_to((P, D)))
            npad_sb = const_p.tile([P, SLOTS], f32)
            nc.sync.dma_start(npad_sb[:], t_npad.ap())
            dginv_sb = const_p.tile([P, SLOTS], f32)
            nc.sync.dma_start(dginv_sb[:], t_dginv.ap())
            nfto_sb = const_p.tile([D + 1, NPAD], bf16)
            nc.sync.dma_start(nfto_sb[:], t_nfto.ap())
            padC_sb = const_p.tile([P, 2 * D], bf16)
            nc.sync.dma_start(padC_sb[:], t_padC.ap())
            Bt_sb = const_p.tile([P, SLOTS, D], bf16)

            # ---- phase 1: C = [A~ | nf] to DRAM; B~ (own, permuted) in SBUF
            with contextlib.ExitStack() as pctx:
                pre_p = pctx.enter_context(tc.tile_pool(name="pre", bufs=3))
                pre_ps = pctx.enter_context(
                    tc.tile_pool(name="preps", bufs=2, space="PSUM"))
                # pad rows at the head of each half
                nc.sync.dma_start(t_C.ap()[0:P, :], padC_sb[:])
                nc.sync.dma_start(t_C.ap()[HR:HR + P, :], padC_sb[:])
                GB = 8
                # compute blocks: rows [128, HR) of each half
                NBH = (HR - P) // P  # 196
                for h in (0, 1):
                    base = h * HR + P
                    for g in range((NBH + GB - 1) // GB):
                        nb = min(GB, NBH - g * GB)
                        c0 = base + g * GB * P
                        xt = pre_p.tile([D + 1, GB * P], bf16, tag="xt")
                        nc.sync.dma_start(
                            xt[:, :nb * P], t_nftC.ap()[:, c0:c0 + nb * P])
                        ps = pre_ps.tile([P, GB, D], f32, tag="ps")
                        for j in range(nb):
                            nc.tensor.matmul(
                                out=ps[:, j, :], lhsT=xt[:, j * P:(j + 1) * P],
                                rhs=w1_sb[:], start=True, stop=True)
                        cb = pre_p.tile([P, GB, 2 * D], bf16, tag="cb")
                        nc.scalar.copy(cb[:, :nb, :D], ps[:, :nb, :])
                        nc.sync.dma_start(
                            cb[:, :nb, D:],
                            t_nfbC.ap()[c0:c0 + nb * P, :].rearrange(
                                "(b p) d -> p b d", p=P))
                        nc.sync.dma_start(
                            t_C.ap()[c0:c0 + nb * P, :].rearrange(
                                "(b p) d -> p b d", p=P), cb[:, :nb, :])
                # B~ for own nodes in permuted (slot, partition) order
                for g in range((SLOTS + GB - 1) // GB):
                    nb = min(GB, SLOTS - g * GB)
                    c0 = g * GB * P
                    ps = pre_ps.tile([P, GB, D], f32, tag="bps")
                    for j in range(nb):
                        nc.tensor.matmul(
                            out=ps[:, j, :],
                            lhsT=nfto_sb[:, c0 + j * P:c0 + (j + 1) * P],
                            rhs=w2_sb[:], start=True, stop=True)
                    nc.scalar.copy(
                        Bt_sb[:, g * GB:g * GB + nb, :], ps[:, :nb, :])

            # ---- phase 2: per-slot edge processing
            i_p = ctx.enter_context(tc.tile_pool(name="idx", bufs=8))
            g_p = ctx.enter_context(tc.tile_pool(name="gath", bufs=6))
            y_p = ctx.enter_context(tc.tile_pool(name="ybuf", bufs=3))
            z_p = ctx.enter_context(tc.tile_pool(name="zbuf", bufs=3))
            pay_p = ctx.enter_context(tc.tile_pool(name="pay", bufs=3))
            s_p = ctx.enter_context(tc.tile_pool(name="small", bufs=4))
            fin_p = ctx.enter_context(tc.tile_pool(name="fin", bufs=3))
            fin_ps = ctx.enter_context(
                tc.tile_pool(name="finps", bufs=2, space="PSUM"))

            for s in range(SLOTS):
                k0, k1 = K0[s], K1[s]
                k = k0 + k1
                gC = g_p.tile([P, KMAX, 2 * D], bf16, tag="gC")
                if k0 > 0:
                    i0 = i_p.tile([P, KM0 * 8], i16, tag="i0")
                    nc.sync.dma_start(
                        i0[:, :k0 * 8],
                        t_i0.ap()[:, int(OFF0[s]):int(OFF0[s]) + k0 * 8])
                    nc.gpsimd.dma_gather(
                        out_ap=gC[:, :k0, :], in_ap=t_C.ap()[:HR, :],
                        idxs_ap=i0[:, :k0 * 8], num_idxs=k0 * P,
                        num_idxs_reg=k0 * P, elem_size=2 * D,
                        single_packet=False, queue_num=(2 * s) % 4)
                if k1 > 0:
                    i1 = i_p.tile([P, KM1 * 8], i16, tag="i1")
                    nc.sync.dma_start(
                        i1[:, :k1 * 8],
                        t_i1.ap()[:, int(OFF1[s]):int(OFF1[s]) + k1 * 8])
                    nc.gpsimd.dma_gather(
                        out_ap=gC[:, k0:k, :], in_ap=t_C.ap()[HR:, :],
                        idxs_ap=i1[:, :k1 * 8], num_idxs=k1 * P,
                        num_idxs_reg=k1 * P, elem_size=2 * D,
                        single_packet=False, queue_num=(2 * s + 1) % 4)

                # y = A~[src] + B~[dst]  (dst == partition's own node)
                y = y_p.tile([P, KMAX, D], bf16, tag="y")
                nc.vector.tensor_tensor(
                    out=y[:, :k, :], in0=gC[:, :k, :D],
                    in1=Bt_sb[:, s, :].unsqueeze(1).broadcast_to((P, k, D)),
                    op=_ALU.add)
                # z: positives-first clamp on ACT engine
                z = z_p.tile([P, KMAX, D], bf16, tag="z")
                if KP > 0:
                    nc.scalar.activation(z[:, :k, :KP], y[:, :k, :KP],
                                         func=_AFT.Relu)
                if KP < D:
                    nc.scalar.activation(z[:, :k, KP:], y[:, :k, KP:],
                                         func=_AFT.Relu, scale=-1.0)
                # logits = sum(z_pos) - sum(z_negstore)
                lg = s_p.tile([P, KMAX], f32, tag="lg")
                if KP > 0 and KP < D:
                    r1 = s_p.tile([P, KMAX], f32, tag="r1")
                    nc.vector.tensor_reduce(lg[:, :k], z[:, :k, :KP],
                                            axis=_ALT.X, op=_ALU.add)
                    nc.vector.tensor_reduce(r1[:, :k], z[:, :k, KP:],
                                            axis=_ALT.X, op=_ALU.add)
                    nc.vector.tensor_tensor(out=lg[:, :k], in0=lg[:, :k],
                                            in1=r1[:, :k], op=_ALU.subtract)
                elif KP == D:
                    nc.vector.tensor_reduce(lg[:, :k], z[:, :k, :],
                                            axis=_ALT.X, op=_ALU.add)
                else:
                    nc.vector.tensor_reduce(lg[:, :k], z[:, :k, :],
                                            axis=_ALT.X, op=_ALU.add)
                    nc.vector.tensor_scalar_mul(lg[:, :k], lg[:, :k], -1.0)
                # wt = exp(leaky_relu(lg))
                lk = s_p.tile([P, KMAX], f32, tag="lk")
                nc.scalar.activation(lk[:, :k], lg[:, :k], func=_AFT.Lrelu,
                                     alpha=0.01)
                wt = s_p.tile([P, KMAX], bf16, tag="wt")
                nc.scalar.activation(wt[:, :k], lk[:, :k], func=_AFT.Exp)

                # pay = nf[src] * wt ; tree-reduce over k -> agg
                pay = pay_p.tile([P, KMAX, D], f32, tag="pay")
                nc.vector.tensor_tensor(
                    out=pay[:, :k, :], in0=gC[:, :k, D:],
                    in1=wt[:, :k].unsqueeze(2).broadcast_to((P, k, D)),
                    op=_ALU.mult)
                m = k
                while m > 1:
                    a = m // 2
                    nc.vector.tensor_tensor(
                        out=pay[:, :a, :], in0=pay[:, :a, :],
                        in1=pay[:, m - a:m, :], op=_ALU.add)
                    m -= a

                # wsum (pad-corrected), normalize
                ws = fin_p.tile([P, 1], f32, tag="ws")
                nc.vector.tensor_reduce(ws[:], wt[:, :k], axis=_ALT.X,
                                        op=_ALU.add)
                nc.vector.tensor_tensor(out=ws[:], in0=ws[:],
                                        in1=npad_sb[:, s:s + 1],
                                        op=_ALU.subtract)
                den = fin_p.tile([P, 1], f32, tag="den")
                nc.vector.tensor_scalar_max(den[:], ws[:], 1e-30)
                rec = fin_p.tile([P, 1], f32, tag="rec")
                nc.vector.reciprocal(rec[:], den[:])
                rec2 = fin_p.tile([P, 1], f32, tag="rec2")
                nc.vector.tensor_mul(rec2[:], rec[:], dginv_sb[:, s:s + 1])
                am = fin_p.tile([P, D], f32, tag="am")
                nc.vector.tensor_mul(
                    am[:], pay[:, 0, :], rec2[:].broadcast_to((P, D)))

                # finale: out = [nf_own | agg] @ Wn + bn
                amT_ps = fin_ps.tile([D, P], f32, tag="amT")
                nc.tensor.transpose(out=amT_ps[:], in_=am[:], identity=ident[:])
                amT = fin_p.tile([D, P], bf16, tag="amTs")
                nc.vector.tensor_copy(amT[:], amT_ps[:])
                o_ps = fin_ps.tile([P, D], f32, tag="ops")
                nc.tensor.matmul(out=o_ps[:],
                                 lhsT=nfto_sb[:D, s * P:(s + 1) * P],
                                 rhs=wn1_sb[:], start=True, stop=False)
                nc.tensor.matmul(out=o_ps[:], lhsT=amT[:], rhs=wn2_sb[:],
                                 start=False, stop=True)
                o_sb = fin_p.tile([P, D], f32, tag="osb")
                nc.vector.tensor_add(o_sb[:], o_ps[:], bn_b[:])
                nc.sync.dma_start(t_out.ap()[s * P:(s + 1) * P, :], o_sb[:])

    nc.compile()
    return nc


_PROGRAM_CACHE = {}


def _get_program(cfg_key, cfg):
    if cfg_key not in _PROGRAM_CACHE:
        _PROGRAM_CACHE[cfg_key] = _build_program(cfg)
    return _PROGRAM_CACHE[cfg_key]


def _prep(nf, src, dst, W_attn, b_attn, w_fc, W_node, b_node, n_cores=8):
    from ml_dtypes import bfloat16

    N = nf.shape[0]
    assert N == N_NODES
    src = np.asarray(src).astype(np.int64)
    dst = np.asarray(dst).astype(np.int64)
    nf = np.asarray(nf, dtype=np.float32)
    W_attn = np.asarray(W_attn, dtype=np.float32)
    b_attn = np.asarray(b_attn, dtype=np.float32)
    w_fc = np.asarray(w_fc, dtype=np.float32)
    W_node = np.asarray(W_node, dtype=np.float32)
    b_node = np.asarray(b_node, dtype=np.float32)

    # hidden-dim permutation: positive wfc dims first; fold wfc into W_attn
    permh = np.argsort(w_fc < 0, kind="stable")
    wfcP = w_fc[permh]
    KP = int((wfcP >= 0).sum())
    w1t = np.concatenate([W_attn[:D], b_attn[None, :]], 0)[:, permh] * wfcP
    w2t = np.concatenate([W_attn[D:], np.zeros((1, D), np.float32)], 0)[
        :, permh] * wfcP

    half = (src >= SPLIT).astype(np.int64)
    src_local = np.where(half == 0, src + P, src - SPLIT + P)

    deg = np.bincount(dst, minlength=N)
    h0 = np.bincount(dst[half == 0], minlength=N)
    h1 = deg - h0

    owner = dst // NPC

    # per-core node packing (perm: rank -> local node id)
    perms = []
    pos_s = np.zeros(N, np.int64)
    pos_p = np.zeros(N, np.int64)
    for c in range(n_cores):
        a = h0[c * NPC:(c + 1) * NPC]
        b = h1[c * NPC:(c + 1) * NPC]
        order = np.lexsort((-b, -a))  # rank -> local node
        # window re-sort by h1 to tighten per-slot h1 maxima
        W = 256
        ob = np.zeros(NPAD, np.int64)
        ob[:NPC] = b[order]
        order_p = np.full(NPAD, -1, np.int64)
        order_p[:NPC] = order
        for w0 in range(0, NPAD, W):
            sl = slice(w0, w0 + W)
            o2 = np.argsort(-ob[sl], kind="stable")
            order_p[sl] = order_p[sl][o2]
            ob[sl] = ob[sl][o2]
        order = order_p[order_p >= 0]
        perms.append(order)
        nodes = c * NPC + order
        ranks = np.arange(NPC)
        pos_s[nodes] = ranks // P
        pos_p[nodes] = ranks % P

    s_e = pos_s[dst]
    p_e = pos_p[dst]

    # per (core, slot, partition, half) counts -> K0/K1 (shared across cores)
    cnt = np.zeros((n_cores, SLOTS, P, 2), np.int64)
    np.add.at(cnt, (owner, s_e, p_e, half), 1)
    K0 = cnt[:, :, :, 0].max(axis=(0, 2))
    K1 = cnt[:, :, :, 1].max(axis=(0, 2))
    K0 = np.maximum(K0, 1).astype(np.int64)
    K1 = np.maximum(K1, 1).astype(np.int64)

    # rank of each edge within its (dst, half) group
    key = ((owner * SLOTS + s_e) * P + p_e) * 2 + half
    order_e = np.argsort(key, kind="stable")
    skey = key[order_e]
    starts = np.searchsorted(skey, np.arange(n_cores * SLOTS * P * 2 + 1))
    rank_in_grp = np.arange(len(src)) - starts[skey]

    OFF0 = np.concatenate([[0], np.cumsum(K0)])
    OFF1 = np.concatenate([[0], np.cumsum(K1)])
    T0, T1 = int(OFF0[-1]), int(OFF1[-1])

    # flat per-core idx vectors (0 = pad row)
    v0 = np.zeros((n_cores, T0 * P), np.int16)
    v1 = np.zeros((n_cores, T1 * P), np.int16)
    oe_src = src_local[order_e]
    oe_half = half[order_e]
    oe_owner = owner[order_e]
    oe_s = s_e[order_e]
    oe_p = p_e[order_e]
    m0 = oe_half == 0
    pos0 = (OFF0[oe_s[m0]] + rank_in_grp[m0]) * P + oe_p[m0]
    v0[oe_owner[m0], pos0] = oe_src[m0]
    m1 = ~m0
    pos1 = (OFF1[oe_s[m1]] + rank_in_grp[m1]) * P + oe_p[m1]
    v1[oe_owner[m1], pos1] = oe_src[m1]

    # pad counts / deg inverse per (p, s), permuted order
    npad = (K0[None, :, None] - cnt[:, :, :, 0]
            + K1[None, :, None] - cnt[:, :, :, 1]).astype(np.float32)
    npad = npad.transpose(0, 2, 1)  # [core, p, s]

    cfg = {"K0": tuple(int(x) for x in K0), "K1": tuple(int(x) for x in K1),
           "KP": KP}

    # C-table-ordered node features
    NFC = 2 * HR
    nftC = np.zeros((D + 1, NFC), np.float32)
    nftC[D, :] = 1.0
    nftC[:D, P:P + SPLIT] = nf[:SPLIT].T
    nftC[:D, HR + P:HR + P + (N - SPLIT)] = nf[SPLIT:].T
    nfbC = np.zeros((NFC, D), np.float32)
    nfbC[P:P + SPLIT] = nf[:SPLIT]
    nfbC[HR + P:HR + P + (N - SPLIT)] = nf[SPLIT:]
    padC = np.zeros((P, 2 * D), np.float32)
    padC[:, :KP] = -3000.0
    padC[:, KP:D] = 3000.0

    in_maps = []
    for c in range(n_cores):
        order = perms[c]
        nodes = c * NPC + order
        nfto = np.zeros((D + 1, NPAD), np.float32)
        nfto[D, :] = 1.0
        nfto[:D, :NPC] = nf[nodes].T
        degc = np.ones(NPAD, np.float32)
        degc[:NPC] = np.maximum(deg[nodes], 1.0)
        dginv = np.ascontiguousarray(
            (1.0 / degc).reshape(SLOTS, P).T).astype(np.float32)
        i0 = np.concatenate(
            [_wrap16(v0[c, OFF0[s] * P:OFF0[s + 1] * P])
             for s in range(SLOTS)], axis=1)
        i1 = np.concatenate(
            [_wrap16(v1[c, OFF1[s] * P:OFF1[s + 1] * P])
             for s in range(SLOTS)], axis=1)
        in_maps.append({
            "nftC": np.ascontiguousarray(nftC).astype(bfloat16),
            "nfbC": np.ascontiguousarray(nfbC).astype(bfloat16),
            "padC": padC.astype(bfloat16),
            "nfto": np.ascontiguousarray(nfto).astype(bfloat16),
            "w1": np.ascontiguousarray(w1t).astype(bfloat16),
            "w2": np.ascontiguousarray(w2t).astype(bfloat16),
            "wn1": np.ascontiguousarray(W_node[:D]).astype(bfloat16),
            "wn2": np.ascontiguousarray(W_node[D:]).astype(bfloat16),
            "bn": b_node.astype(np.float32),
            "i0": i0, "i1": i1,
            "npad": np.ascontiguousarray(npad[c]),
            "dginv": dginv,
        })
    return cfg, in_maps, perms


def _run(inputs, trace=False):
    import concourse.bass_utils as bass_utils

    cfg, in_maps, perms = _prep(**inputs)
    cfg_key = (cfg["K0"], cfg["K1"], cfg["KP"])
    nc = _get_program(cfg_key, cfg)
    res = bass_utils.run_bass_kernel_spmd(nc, in_maps,
                                          core_ids=list(range(8)),
                                          trace=trace)
    out = np.empty((N_NODES, D), np.float32)
    for c in range(8):
        rows = np.asarray(res.results[c]["out"][:NPC], np.float32)
        out[c * NPC + perms[c]] = rows
    return out, res


def kernel(nf, src, dst, W_attn, b_attn, w_fc, W_node, b_node):
    out, _ = _run(dict(nf=nf, src=src, dst=dst, W_attn=W_attn, b_attn=b_attn,
                       w_fc=w_fc, W_node=W_node, b_node=b_node))
    return out
